# revision 17
# baseline (speedup 1.0000x reference)
import sys

if "/opt/trn_rl_repo" not in sys.path:
    sys.path.insert(0, "/opt/trn_rl_repo")

import numpy as np

import concourse.mybir as mybir
import concourse.tile as tile
from concourse import bacc, bass_utils
from concourse.masks import make_identity

N_CORES = 8
B, IN, H, OUT = 16384, 784, 4096, 10
BN_EPS = 1e-5
KFULL = 6
KF = KFULL * 128
KTAIL = IN - KF

f32 = mybir.dt.float32
bf16 = mybir.dt.bfloat16
f16 = mybir.dt.float16
AF = mybir.ActivationFunctionType
ALU = mybir.AluOpType


def build_nc(b_sh=B // N_CORES, h_dim=H, n_cores=N_CORES, use_collective=True,
             group_size=4, repeats=1):
    nm = h_dim // 128
    nbt = b_sh // 128
    ngrp = nm // group_size
    batch_total = b_sh * n_cores if use_collective else b_sh

    nc = bacc.Bacc("TRN2", target_bir_lowering=False, debug=False,
                   num_devices=n_cores)

    x_in = nc.dram_tensor("x", [b_sh, IN], f32, kind="ExternalInput").ap()
    w1_in = nc.dram_tensor("W1", [h_dim, IN], f32, kind="ExternalInput").ap()
    gamma_in = nc.dram_tensor("gamma", [h_dim], f32, kind="ExternalInput").ap()
    beta_in = nc.dram_tensor("beta", [h_dim], f32, kind="ExternalInput").ap()
    w2_in = nc.dram_tensor("W2", [OUT, h_dim], f32, kind="ExternalInput").ap()
    out_d = nc.dram_tensor("out", [b_sh, OUT], f32, kind="ExternalOutput").ap()

    with tile.TileContext(nc) as tc:
        for _rep in range(repeats):
            _emit(nc, tc, _rep, x_in, w1_in, gamma_in, beta_in, w2_in, out_d,
                  b_sh, h_dim, n_cores, nm, nbt, ngrp, group_size,
                  batch_total, use_collective)

    nc.compile()
    return nc


def _emit(nc, tc, rep, x_in, w1_in, gamma_in, beta_in, w2_in, out_d,
          b_sh, h_dim, n_cores, nm, nbt, ngrp, gs, batch_total,
          use_collective):
    with (
        tc.tile_pool(name=f"r{rep}const", bufs=1) as const,
        tc.tile_pool(name=f"r{rep}dram", bufs=1, space="DRAM") as dram,
    ):
        ident = const.tile([128, 128], f32)
        make_identity(nc, ident[:])
        ident16 = const.tile([128, 128], f16)
        nc.vector.tensor_copy(ident16[:], ident[:])
        identb = const.tile([128, 128], bf16)
        nc.vector.tensor_copy(identb[:], ident[:])
        sW2T = const.tile([128, nm, OUT], bf16)
        gamma_pm = const.tile([128, nm], f32)
        beta_pm = const.tile([128, nm], f32)
        scale_pm = const.tile([128, nm], f32)
        bias_pm = const.tile([128, nm], f32)
        stats = const.tile([128, nm, 4], f32)
        nc.vector.memset(stats[:], 0.0)

        w1bf_d = dram.tile([h_dim, KF + 128], bf16)

        with tc.tile_pool(name=f"r{rep}persist", bufs=1) as persist:
            xhiT = [persist.tile([128, b_sh], f16, name=f"xhiT{k}")
                    for k in range(KFULL)]
            xloT = [persist.tile([128, b_sh], bf16, name=f"xloT{k}")
                    for k in range(KFULL)]
            xmixT = persist.tile([128, b_sh], f16)
            sW1T = [persist.tile([128, h_dim], bf16, name=f"sW1T{k}")
                    for k in range(KFULL)]
            sW1mixT = persist.tile([128, h_dim], bf16)

            with (
                tc.tile_pool(name=f"r{rep}prolog", bufs=2) as prolog,
                tc.tile_pool(name=f"r{rep}prolog1", bufs=1) as prolog1,
                tc.tile_pool(name=f"r{rep}pps", bufs=4, space="PSUM") as pps,
            ):
                w2_sb = prolog1.tile([OUT, h_dim], f32, tag="w2sb")
                nc.scalar.dma_start(w2_sb[:], w2_in)
                for m in range(nm):
                    pt = pps.tile([128, OUT], f32, tag="pp")
                    nc.tensor.transpose(
                        pt[:], w2_sb[:OUT, m * 128:(m + 1) * 128],
                        ident[:OUT, :OUT])
                    nc.scalar.activation(sW2T[:, m, :], pt[:], AF.Sign)

                ga_sb = prolog1.tile([nm, 128], f32, tag="gasb")
                be_sb = prolog1.tile([nm, 128], f32, tag="besb")
                nc.scalar.dma_start(
                    ga_sb[:], gamma_in.rearrange("(m p) -> m p", p=128))
                nc.scalar.dma_start(
                    be_sb[:], beta_in.rearrange("(m p) -> m p", p=128))
                ga_ps = pps.tile([128, nm], f32, tag="pp")
                nc.tensor.transpose(ga_ps[:], ga_sb[:], ident[:nm, :nm])
                nc.scalar.copy(gamma_pm[:], ga_ps[:])
                be_ps = pps.tile([128, nm], f32, tag="pp")
                nc.tensor.transpose(be_ps[:], be_sb[:], ident[:nm, :nm])
                nc.scalar.copy(beta_pm[:], be_ps[:])

                NQ = 4
                xq = nbt // NQ
                wq = nm // NQ
                for q in range(NQ):
                    xt = prolog.tile([128, xq, IN], f32, tag="xt")
                    nc.gpsimd.dma_start(
                        xt[:],
                        x_in[q * xq * 128:(q + 1) * xq * 128, :].rearrange(
                            "(t p) c -> p t c", p=128))
                    xhi = prolog.tile([128, xq, KF + 128], f16, tag="xhi")
                    xlo = prolog.tile([128, xq, KF], bf16, tag="xlo")
                    nc.vector.tensor_copy(xhi[:, :, :IN], xt[:])
                    nc.vector.tensor_tensor(
                        xlo[:], xt[:, :, :KF], xhi[:, :, :KF],
                        op=ALU.subtract)
                    nc.vector.tensor_tensor(
                        xhi[:, :, IN:IN + KTAIL], xt[:, :, KF:],
                        xhi[:, :, KF:IN], op=ALU.subtract)
                    nc.vector.memset(xhi[:, :, IN + KTAIL:], 0.0)
                    for ti in range(xq):
                        t = q * xq + ti
                        tcol = slice(t * 128, (t + 1) * 128)
                        for k in range(KFULL + 1):
                            pth = pps.tile([128, 128], f16, tag="pp")
                            nc.tensor.transpose(
                                pth[:], xhi[:, ti, k * 128:(k + 1) * 128],
                                ident16[:])
                            dst = xmixT if k == KFULL else xhiT[k]
                            nc.vector.tensor_copy(dst[:, tcol], pth[:])
                        for k in range(KFULL):
                            ptl = pps.tile([128, 128], bf16, tag="pp")
                            nc.tensor.transpose(
                                ptl[:], xlo[:, ti, k * 128:(k + 1) * 128],
                                identb[:])
                            nc.vector.tensor_copy(xloT[k][:, tcol], ptl[:])

                    wr = slice(q * wq * 128, (q + 1) * wq * 128)
                    nc.gpsimd.dma_start(w1bf_d[wr, :IN], w1_in[wr, :])
                    for k in range(KFULL):
                        nc.scalar.dma_start_transpose(
                            sW1T[k][:, wr], w1bf_d[wr, k * 128:(k + 1) * 128])
                    nc.scalar.dma_start_transpose(
                        sW1mixT[:, wr], w1bf_d[wr, KF:])

                nc.sync.dma_start(sW1mixT[16:32, :], sW1mixT[0:16, :])
                for wtile in sW1T:
                    nc.vector.tensor_scalar(
                        wtile[:], wtile[:], 0.0, None, op0=ALU.is_ge)
                    nc.vector.tensor_scalar(
                        wtile[:], wtile[:], 2.0, 1.0,
                        op0=ALU.mult, op1=ALU.subtract)
                nc.vector.tensor_scalar(
                    sW1mixT[0:32, :], sW1mixT[0:32, :], 0.0, None,
                    op0=ALU.is_ge)
                nc.vector.tensor_scalar(
                    sW1mixT[0:32, :], sW1mixT[0:32, :], 2.0, 1.0,
                    op0=ALU.mult, op1=ALU.subtract)
                nc.vector.memset(sW1mixT[32:64, :], 0.0)
                nc.vector.memset(sW1mixT[64:96, :], 0.0)
                nc.vector.memset(sW1mixT[96:128, :], 0.0)

            with (
                tc.tile_pool(name=f"r{rep}hwin", bufs=gs + 3) as hwin,
                tc.tile_pool(name=f"r{rep}sq", bufs=2) as sqp,
                tc.tile_pool(name=f"r{rep}sg", bufs=2) as sgp,
                tc.tile_pool(name=f"r{rep}gst", bufs=2) as gstp,
                tc.tile_pool(name=f"r{rep}ps1", bufs=2, space="PSUM") as ps1,
                tc.tile_pool(name=f"r{rep}ps2", bufs=1, space="PSUM") as ps2,
                tc.tile_pool(name=f"r{rep}ep", bufs=1) as ep,
            ):
                psL = ps2.tile([OUT, b_sh], f32, tag="psl")
                passes = (
                    [(sW1T[k], xhiT[k]) for k in range(KFULL)]
                    + [(sW1T[k], xloT[k]) for k in range(KFULL)]
                    + [(sW1mixT, xmixT)]
                )
                h_tiles = {}

                hsz = min(1024, b_sh)
                ncs = max(1, hsz // 512)
                csz = hsz // ncs
                for g in range(ngrp):
                    for m in range(g * gs, (g + 1) * gs):
                        h_sb = hwin.tile([128, b_sh], f32, tag="hsb")
                        h_tiles[m] = h_sb
                        for hf in range(b_sh // hsz):
                            ph = ps1.tile([128, hsz], f32, tag="ph")
                            for pi, (wt, xt_) in enumerate(passes):
                                lhsT = wt[:, m * 128:(m + 1) * 128]
                                for c in range(ncs):
                                    off = hf * hsz + c * csz
                                    nc.tensor.matmul(
                                        ph[:, c * csz:(c + 1) * csz],
                                        lhsT, xt_[:, off:off + csz],
                                        start=(pi == 0),
                                        stop=(pi == len(passes) - 1),
                                    )
                            nc.scalar.activation(
                                h_sb[:, hf * hsz:(hf + 1) * hsz], ph[:],
                                AF.Identity,
                                accum_out=stats[:, m, hf:hf + 1])
                            sq = sqp.tile([128, hsz], bf16, tag="sq")
                            nc.scalar.activation(
                                sq[:], ph[:], AF.Square,
                                accum_out=stats[:, m, 2 + hf:3 + hf])

                    c_in = dram.tile([128, gs * 4], f32, name=f"cci{g}")
                    c_out = dram.tile([128, gs * 4], f32, name=f"cco{g}")
                    nc.sync.dma_start(
                        c_in[:], stats[:, g * gs:(g + 1) * gs, :])
                    if use_collective:
                        nc.gpsimd.collective_compute(
                            "AllReduce", ALU.add,
                            replica_groups=[list(range(n_cores))],
                            ins=[c_in.opt()], outs=[c_out.opt()],
                        )
                    else:
                        nc.sync.dma_start(c_out[:], c_in[:])
                    gst = gstp.tile([128, gs, 4], f32, tag="gst")
                    nc.sync.dma_start(gst[:], c_out[:])

                    msl = slice(g * gs, (g + 1) * gs)
                    mean_t = gstp.tile([128, gs], f32, tag="mean")
                    var_t = gstp.tile([128, gs], f32, tag="var")
                    tmp_t = gstp.tile([128, gs], f32, tag="tmp")
                    nc.vector.tensor_tensor(
                        mean_t[:], gst[:, :, 0], gst[:, :, 1], op=ALU.add)
                    nc.vector.tensor_scalar_mul(
                        mean_t[:], mean_t[:], 1.0 / batch_total)
                    nc.vector.tensor_tensor(
                        var_t[:], gst[:, :, 2], gst[:, :, 3], op=ALU.add)
                    nc.vector.tensor_scalar_mul(
                        var_t[:], var_t[:], 1.0 / batch_total)
                    nc.vector.tensor_tensor(
                        tmp_t[:], mean_t[:], mean_t[:], op=ALU.mult)
                    nc.vector.tensor_tensor(
                        var_t[:], var_t[:], tmp_t[:], op=ALU.subtract)
                    nc.vector.tensor_scalar_add(var_t[:], var_t[:], BN_EPS)
                    nc.vector.reciprocal(tmp_t[:], var_t[:])
                    nc.scalar.activation(tmp_t[:], tmp_t[:], AF.Sqrt)
                    nc.vector.tensor_tensor(
                        scale_pm[:, msl], tmp_t[:], gamma_pm[:, msl],
                        op=ALU.mult)
                    nc.vector.tensor_tensor(
                        tmp_t[:], mean_t[:], scale_pm[:, msl], op=ALU.mult)
                    nc.vector.tensor_tensor(
                        bias_pm[:, msl], beta_pm[:, msl], tmp_t[:],
                        op=ALU.subtract)

                    for m in range(g * gs, (g + 1) * gs):
                        s_t = sgp.tile([128, b_sh], bf16, tag="st")
                        nc.scalar.activation(
                            s_t[:], h_tiles.pop(m)[:], AF.Sign,
                            bias=bias_pm[:, m:m + 1],
                            scale=scale_pm[:, m:m + 1])
                        for c in range(b_sh // 512):
                            nc.tensor.matmul(
                                psL[:, c * 512:(c + 1) * 512],
                                sW2T[:, m:m + 1, :],
                                s_t[:, c * 512:(c + 1) * 512],
                                start=(m == 0), stop=(m == nm - 1),
                            )

                LT = ep.tile([OUT, b_sh], f32)
                nc.scalar.copy(LT[:], psL[:])
                psT = ps2.tile([128, nbt * OUT], f32, tag="psl")
                for t in range(nbt):
                    nc.tensor.transpose(
                        psT[:, t * OUT:(t + 1) * OUT],
                        LT[:OUT, t * 128:(t + 1) * 128],
                        ident[:OUT, :OUT])
                Lb = ep.tile([128, nbt, OUT], f32)
                nc.scalar.copy(Lb[:], psT[:])

                negmax = ep.tile([128, nbt], f32)
                nc.vector.tensor_reduce(
                    negmax[:], Lb[:], axis=mybir.AxisListType.X,
                    op=ALU.max, negate=True)
                shifted = ep.tile([128, nbt, OUT], f32)
                nc.vector.tensor_tensor(
                    shifted[:], Lb[:],
                    negmax[:][:, :, None].broadcast_to([128, nbt, OUT]),
                    op=ALU.add)
                expv = ep.tile([128, nbt, OUT], f32)
                nc.scalar.activation(expv[:], shifted[:], AF.Exp)
                sumexp = ep.tile([128, nbt], f32)
                nc.vector.tensor_reduce(
                    sumexp[:], expv[:], axis=mybir.AxisListType.X, op=ALU.add)
                lse = ep.tile([128, nbt], f32)
                nc.scalar.activation(lse[:], sumexp[:], AF.Ln)
                lsm = ep.tile([128, nbt, OUT], f32)
                nc.vector.tensor_tensor(
                    lsm[:], shifted[:],
                    lse[:][:, :, None].broadcast_to([128, nbt, OUT]),
                    op=ALU.subtract)
                nc.sync.dma_start(
                    out_d.rearrange("(t p) o -> p t o", p=128), lsm[:])


_NC_CACHE = {}


def _get_nc():
    if "nc" not in _NC_CACHE:
        _NC_CACHE["nc"] = build_nc()
    return _NC_CACHE["nc"]


def kernel(x, W1, gamma, beta, W2):
    x = np.ascontiguousarray(np.asarray(x), dtype=np.float32)
    W1 = np.ascontiguousarray(np.asarray(W1), dtype=np.float32)
    gamma = np.ascontiguousarray(np.asarray(gamma), dtype=np.float32)
    beta = np.ascontiguousarray(np.asarray(beta), dtype=np.float32)
    W2 = np.ascontiguousarray(np.asarray(W2), dtype=np.float32)

    nc = _get_nc()
    b_sh = B // N_CORES
    in_maps = [
        {
            "x": x[c * b_sh:(c + 1) * b_sh],
            "W1": W1,
            "gamma": gamma,
            "beta": beta,
            "W2": W2,
        }
        for c in range(N_CORES)
    ]
    res = bass_utils.run_bass_kernel_spmd(
        nc, in_maps, core_ids=list(range(N_CORES)))
    return np.concatenate(
        [res.results[c]["out"] for c in range(N_CORES)], axis=0)


# revision 19
# speedup vs baseline: 1.0427x; 1.0427x over previous
import sys

if "/opt/trn_rl_repo" not in sys.path:
    sys.path.insert(0, "/opt/trn_rl_repo")

import numpy as np

import concourse.mybir as mybir
import concourse.tile as tile
from concourse import bacc, bass_utils
from concourse.masks import make_identity

N_CORES = 8
B, IN, H, OUT = 16384, 784, 4096, 10
BN_EPS = 1e-5
KFULL = 6
KF = KFULL * 128
KTAIL = IN - KF

f32 = mybir.dt.float32
bf16 = mybir.dt.bfloat16
f16 = mybir.dt.float16
AF = mybir.ActivationFunctionType
ALU = mybir.AluOpType


def build_nc(b_sh=B // N_CORES, h_dim=H, n_cores=N_CORES, use_collective=True,
             group_size=4, repeats=1):
    nm = h_dim // 128
    nbt = b_sh // 128
    groups = []
    mstart = 0
    while mstart < nm:
        g_sz = min(group_size, nm - mstart)
        if nm - mstart == group_size and group_size >= 4:
            groups.append(list(range(mstart, mstart + g_sz // 2)))
            groups.append(list(range(mstart + g_sz // 2, mstart + g_sz)))
        else:
            groups.append(list(range(mstart, mstart + g_sz)))
        mstart += g_sz
    batch_total = b_sh * n_cores if use_collective else b_sh

    nc = bacc.Bacc("TRN2", target_bir_lowering=False, debug=False,
                   num_devices=n_cores)

    x_in = nc.dram_tensor("x", [b_sh, IN], f32, kind="ExternalInput").ap()
    w1_in = nc.dram_tensor("W1", [h_dim, IN], f32, kind="ExternalInput").ap()
    gamma_in = nc.dram_tensor("gamma", [h_dim], f32, kind="ExternalInput").ap()
    beta_in = nc.dram_tensor("beta", [h_dim], f32, kind="ExternalInput").ap()
    w2_in = nc.dram_tensor("W2", [OUT, h_dim], f32, kind="ExternalInput").ap()
    out_d = nc.dram_tensor("out", [b_sh, OUT], f32, kind="ExternalOutput").ap()

    with tile.TileContext(nc) as tc:
        for _rep in range(repeats):
            _emit(nc, tc, _rep, x_in, w1_in, gamma_in, beta_in, w2_in, out_d,
                  b_sh, h_dim, n_cores, nm, nbt, groups, group_size,
                  batch_total, use_collective)

    nc.compile()
    return nc


def _emit(nc, tc, rep, x_in, w1_in, gamma_in, beta_in, w2_in, out_d,
          b_sh, h_dim, n_cores, nm, nbt, groups, gs, batch_total,
          use_collective):
    with (
        tc.tile_pool(name=f"r{rep}const", bufs=1) as const,
        tc.tile_pool(name=f"r{rep}dram", bufs=1, space="DRAM") as dram,
    ):
        ident = const.tile([128, 128], f32)
        make_identity(nc, ident[:])
        ident16 = const.tile([128, 128], f16)
        nc.vector.tensor_copy(ident16[:], ident[:])
        identb = const.tile([128, 128], bf16)
        nc.vector.tensor_copy(identb[:], ident[:])
        sW2T = const.tile([128, nm, OUT], bf16)
        gamma_pm = const.tile([128, nm], f32)
        beta_pm = const.tile([128, nm], f32)
        scale_pm = const.tile([128, nm], f32)
        bias_pm = const.tile([128, nm], f32)
        stats = const.tile([128, nm, 4], f32)
        nc.vector.memset(stats[:], 0.0)

        w1bf_d = dram.tile([h_dim, KF + 128], bf16)

        with tc.tile_pool(name=f"r{rep}persist", bufs=1) as persist:
            xhiT = [persist.tile([128, b_sh], f16, name=f"xhiT{k}")
                    for k in range(KFULL)]
            xloT = [persist.tile([128, b_sh], bf16, name=f"xloT{k}")
                    for k in range(KFULL)]
            xmixT = persist.tile([128, b_sh], f16)
            sW1T = [persist.tile([128, h_dim], bf16, name=f"sW1T{k}")
                    for k in range(KFULL)]
            sW1mixT = persist.tile([128, h_dim], bf16)

            with (
                tc.tile_pool(name=f"r{rep}prolog", bufs=2) as prolog,
                tc.tile_pool(name=f"r{rep}prolog1", bufs=1) as prolog1,
                tc.tile_pool(name=f"r{rep}pps", bufs=4, space="PSUM") as pps,
            ):
                w2_sb = prolog1.tile([OUT, h_dim], f32, tag="w2sb")
                nc.scalar.dma_start(w2_sb[:], w2_in)
                for m in range(nm):
                    pt = pps.tile([128, OUT], f32, tag="pp")
                    nc.tensor.transpose(
                        pt[:], w2_sb[:OUT, m * 128:(m + 1) * 128],
                        ident[:OUT, :OUT])
                    nc.scalar.activation(sW2T[:, m, :], pt[:], AF.Sign)

                ga_sb = prolog1.tile([nm, 128], f32, tag="gasb")
                be_sb = prolog1.tile([nm, 128], f32, tag="besb")
                nc.scalar.dma_start(
                    ga_sb[:], gamma_in.rearrange("(m p) -> m p", p=128))
                nc.scalar.dma_start(
                    be_sb[:], beta_in.rearrange("(m p) -> m p", p=128))
                ga_ps = pps.tile([128, nm], f32, tag="pp")
                nc.tensor.transpose(ga_ps[:], ga_sb[:], ident[:nm, :nm])
                nc.scalar.copy(gamma_pm[:], ga_ps[:])
                be_ps = pps.tile([128, nm], f32, tag="pp")
                nc.tensor.transpose(be_ps[:], be_sb[:], ident[:nm, :nm])
                nc.scalar.copy(beta_pm[:], be_ps[:])

                NQ = 4
                xq = nbt // NQ
                wq = nm // NQ
                for q in range(NQ):
                    xt = prolog.tile([128, xq, IN], f32, tag="xt")
                    nc.sync.dma_start(
                        xt[:],
                        x_in[q * xq * 128:(q + 1) * xq * 128, :].rearrange(
                            "(t p) c -> p t c", p=128))
                    xhi = prolog.tile([128, xq, KF + 128], f16, tag="xhi")
                    xlo = prolog.tile([128, xq, KF], bf16, tag="xlo")
                    nc.vector.tensor_copy(xhi[:, :, :IN], xt[:])
                    nc.vector.tensor_tensor(
                        xlo[:], xt[:, :, :KF], xhi[:, :, :KF],
                        op=ALU.subtract)
                    nc.vector.tensor_tensor(
                        xhi[:, :, IN:IN + KTAIL], xt[:, :, KF:],
                        xhi[:, :, KF:IN], op=ALU.subtract)
                    nc.vector.memset(xhi[:, :, IN + KTAIL:], 0.0)
                    for ti in range(xq):
                        t = q * xq + ti
                        tcol = slice(t * 128, (t + 1) * 128)
                        for k in range(KFULL + 1):
                            pth = pps.tile([128, 128], f16, tag="pp")
                            nc.tensor.transpose(
                                pth[:], xhi[:, ti, k * 128:(k + 1) * 128],
                                ident16[:])
                            dst = xmixT if k == KFULL else xhiT[k]
                            nc.vector.tensor_copy(dst[:, tcol], pth[:])
                        for k in range(KFULL):
                            ptl = pps.tile([128, 128], bf16, tag="pp")
                            nc.tensor.transpose(
                                ptl[:], xlo[:, ti, k * 128:(k + 1) * 128],
                                identb[:])
                            nc.vector.tensor_copy(xloT[k][:, tcol], ptl[:])

                    wr = slice(q * wq * 128, (q + 1) * wq * 128)
                    nc.gpsimd.dma_start(w1bf_d[wr, :IN], w1_in[wr, :])
                    for k in range(KFULL):
                        nc.scalar.dma_start_transpose(
                            sW1T[k][:, wr], w1bf_d[wr, k * 128:(k + 1) * 128])
                    nc.scalar.dma_start_transpose(
                        sW1mixT[:, wr], w1bf_d[wr, KF:])

                nc.sync.dma_start(sW1mixT[16:32, :], sW1mixT[0:16, :])
                for wtile in sW1T:
                    nc.vector.tensor_scalar(
                        wtile[:], wtile[:], 0.0, None, op0=ALU.is_ge)
                    nc.vector.tensor_scalar(
                        wtile[:], wtile[:], 2.0, 1.0,
                        op0=ALU.mult, op1=ALU.subtract)
                nc.vector.tensor_scalar(
                    sW1mixT[0:32, :], sW1mixT[0:32, :], 0.0, None,
                    op0=ALU.is_ge)
                nc.vector.tensor_scalar(
                    sW1mixT[0:32, :], sW1mixT[0:32, :], 2.0, 1.0,
                    op0=ALU.mult, op1=ALU.subtract)
                nc.vector.memset(sW1mixT[32:64, :], 0.0)
                nc.vector.memset(sW1mixT[64:96, :], 0.0)
                nc.vector.memset(sW1mixT[96:128, :], 0.0)

            with (
                tc.tile_pool(name=f"r{rep}hwin", bufs=gs + 3) as hwin,
                tc.tile_pool(name=f"r{rep}sq", bufs=2) as sqp,
                tc.tile_pool(name=f"r{rep}sg", bufs=2) as sgp,
                tc.tile_pool(name=f"r{rep}gst", bufs=2) as gstp,
                tc.tile_pool(name=f"r{rep}ps1", bufs=2, space="PSUM") as ps1,
                tc.tile_pool(name=f"r{rep}ps2", bufs=1, space="PSUM") as ps2,
                tc.tile_pool(name=f"r{rep}ep", bufs=1) as ep,
            ):
                psL = ps2.tile([OUT, b_sh], f32, tag="psl")
                passes = (
                    [(sW1T[k], xhiT[k]) for k in range(KFULL)]
                    + [(sW1T[k], xloT[k]) for k in range(KFULL)]
                    + [(sW1mixT, xmixT)]
                )
                h_tiles = {}

                hsz = min(1024, b_sh)
                ncs = max(1, hsz // 512)
                csz = hsz // ncs
                for g, gms in enumerate(groups):
                    for m in gms:
                        h_sb = hwin.tile([128, b_sh], f32, tag="hsb")
                        h_tiles[m] = h_sb
                        for hf in range(b_sh // hsz):
                            ph = ps1.tile([128, hsz], f32, tag="ph")
                            for pi, (wt, xt_) in enumerate(passes):
                                lhsT = wt[:, m * 128:(m + 1) * 128]
                                for c in range(ncs):
                                    off = hf * hsz + c * csz
                                    nc.tensor.matmul(
                                        ph[:, c * csz:(c + 1) * csz],
                                        lhsT, xt_[:, off:off + csz],
                                        start=(pi == 0),
                                        stop=(pi == len(passes) - 1),
                                    )
                            nc.scalar.activation(
                                h_sb[:, hf * hsz:(hf + 1) * hsz], ph[:],
                                AF.Identity,
                                accum_out=stats[:, m, hf:hf + 1])
                            sq = sqp.tile([128, hsz], bf16, tag="sq")
                            nc.scalar.activation(
                                sq[:], ph[:], AF.Square,
                                accum_out=stats[:, m, 2 + hf:3 + hf])

                    g0, gn = gms[0], len(gms)
                    c_in = dram.tile([128, gn * 4], f32, name=f"cci{g}")
                    c_out = dram.tile([128, gn * 4], f32, name=f"cco{g}")
                    nc.sync.dma_start(
                        c_in[:], stats[:, g0:g0 + gn, :])
                    if use_collective:
                        nc.gpsimd.collective_compute(
                            "AllReduce", ALU.add,
                            replica_groups=[list(range(n_cores))],
                            ins=[c_in.opt()], outs=[c_out.opt()],
                        )
                    else:
                        nc.sync.dma_start(c_out[:], c_in[:])
                    gst = gstp.tile([128, gn, 4], f32, tag="gst")
                    nc.sync.dma_start(gst[:], c_out[:])

                    msl = slice(g0, g0 + gn)
                    mean_t = gstp.tile([128, gn], f32, tag="mean")
                    var_t = gstp.tile([128, gn], f32, tag="var")
                    tmp_t = gstp.tile([128, gn], f32, tag="tmp")
                    nc.vector.tensor_tensor(
                        mean_t[:], gst[:, :, 0], gst[:, :, 1], op=ALU.add)
                    nc.vector.tensor_scalar_mul(
                        mean_t[:], mean_t[:], 1.0 / batch_total)
                    nc.vector.tensor_tensor(
                        var_t[:], gst[:, :, 2], gst[:, :, 3], op=ALU.add)
                    nc.vector.tensor_scalar_mul(
                        var_t[:], var_t[:], 1.0 / batch_total)
                    nc.vector.tensor_tensor(
                        tmp_t[:], mean_t[:], mean_t[:], op=ALU.mult)
                    nc.vector.tensor_tensor(
                        var_t[:], var_t[:], tmp_t[:], op=ALU.subtract)
                    nc.vector.tensor_scalar_add(var_t[:], var_t[:], BN_EPS)
                    nc.vector.reciprocal(tmp_t[:], var_t[:])
                    nc.scalar.activation(tmp_t[:], tmp_t[:], AF.Sqrt)
                    nc.vector.tensor_tensor(
                        scale_pm[:, msl], tmp_t[:], gamma_pm[:, msl],
                        op=ALU.mult)
                    nc.vector.tensor_tensor(
                        tmp_t[:], mean_t[:], scale_pm[:, msl], op=ALU.mult)
                    nc.vector.tensor_tensor(
                        bias_pm[:, msl], beta_pm[:, msl], tmp_t[:],
                        op=ALU.subtract)

                    for m in gms:
                        s_t = sgp.tile([128, b_sh], bf16, tag="st")
                        nc.scalar.activation(
                            s_t[:], h_tiles.pop(m)[:], AF.Sign,
                            bias=bias_pm[:, m:m + 1],
                            scale=scale_pm[:, m:m + 1])
                        for c in range(b_sh // 512):
                            nc.tensor.matmul(
                                psL[:, c * 512:(c + 1) * 512],
                                sW2T[:, m:m + 1, :],
                                s_t[:, c * 512:(c + 1) * 512],
                                start=(m == 0), stop=(m == nm - 1),
                            )

                LT = ep.tile([OUT, b_sh], f32)
                nc.scalar.copy(LT[:], psL[:])
                psT = ps2.tile([128, nbt * OUT], f32, tag="psl")
                for t in range(nbt):
                    nc.tensor.transpose(
                        psT[:, t * OUT:(t + 1) * OUT],
                        LT[:OUT, t * 128:(t + 1) * 128],
                        ident[:OUT, :OUT])
                Lb = ep.tile([128, nbt, OUT], f32)
                nc.scalar.copy(Lb[:], psT[:])

                negmax = ep.tile([128, nbt], f32)
                nc.vector.tensor_reduce(
                    negmax[:], Lb[:], axis=mybir.AxisListType.X,
                    op=ALU.max, negate=True)
                shifted = ep.tile([128, nbt, OUT], f32)
                nc.vector.tensor_tensor(
                    shifted[:], Lb[:],
                    negmax[:][:, :, None].broadcast_to([128, nbt, OUT]),
                    op=ALU.add)
                expv = ep.tile([128, nbt, OUT], f32)
                nc.scalar.activation(expv[:], shifted[:], AF.Exp)
                sumexp = ep.tile([128, nbt], f32)
                nc.vector.tensor_reduce(
                    sumexp[:], expv[:], axis=mybir.AxisListType.X, op=ALU.add)
                lse = ep.tile([128, nbt], f32)
                nc.scalar.activation(lse[:], sumexp[:], AF.Ln)
                lsm = ep.tile([128, nbt, OUT], f32)
                nc.vector.tensor_tensor(
                    lsm[:], shifted[:],
                    lse[:][:, :, None].broadcast_to([128, nbt, OUT]),
                    op=ALU.subtract)
                nc.sync.dma_start(
                    out_d.rearrange("(t p) o -> p t o", p=128), lsm[:])


_NC_CACHE = {}


def _get_nc():
    if "nc" not in _NC_CACHE:
        _NC_CACHE["nc"] = build_nc()
    return _NC_CACHE["nc"]


def kernel(x, W1, gamma, beta, W2):
    x = np.ascontiguousarray(np.asarray(x), dtype=np.float32)
    W1 = np.ascontiguousarray(np.asarray(W1), dtype=np.float32)
    gamma = np.ascontiguousarray(np.asarray(gamma), dtype=np.float32)
    beta = np.ascontiguousarray(np.asarray(beta), dtype=np.float32)
    W2 = np.ascontiguousarray(np.asarray(W2), dtype=np.float32)

    nc = _get_nc()
    b_sh = B // N_CORES
    in_maps = [
        {
            "x": x[c * b_sh:(c + 1) * b_sh],
            "W1": W1,
            "gamma": gamma,
            "beta": beta,
            "W2": W2,
        }
        for c in range(N_CORES)
    ]
    res = bass_utils.run_bass_kernel_spmd(
        nc, in_maps, core_ids=list(range(N_CORES)))
    return np.concatenate(
        [res.results[c]["out"] for c in range(N_CORES)], axis=0)


# revision 27
# speedup vs baseline: 1.0850x; 1.0406x over previous
import sys

if "/opt/trn_rl_repo" not in sys.path:
    sys.path.insert(0, "/opt/trn_rl_repo")

import numpy as np

import concourse.mybir as mybir
import concourse.tile as tile
from concourse import bacc, bass_utils
from concourse.masks import make_identity

N_CORES = 8
B, IN, H, OUT = 16384, 784, 4096, 10
BN_EPS = 1e-5
KFULL = 6
KF = KFULL * 128
KTAIL = IN - KF

f32 = mybir.dt.float32
bf16 = mybir.dt.bfloat16
f16 = mybir.dt.float16
AF = mybir.ActivationFunctionType
ALU = mybir.AluOpType


def build_nc(b_sh=B // N_CORES, h_dim=H, n_cores=N_CORES, use_collective=True,
             group_size=4, repeats=1):
    nm = h_dim // 128
    nbt = b_sh // 128
    groups = []
    mstart = 0
    while mstart < nm:
        g_sz = min(group_size, nm - mstart)
        if nm - mstart == group_size and group_size >= 4:
            groups.append(list(range(mstart, mstart + g_sz // 2)))
            groups.append(list(range(mstart + g_sz // 2, mstart + g_sz)))
        else:
            groups.append(list(range(mstart, mstart + g_sz)))
        mstart += g_sz
    batch_total = b_sh * n_cores if use_collective else b_sh

    nc = bacc.Bacc("TRN2", target_bir_lowering=False, debug=False,
                   num_devices=n_cores)

    x_in = nc.dram_tensor("x", [b_sh, IN], f32, kind="ExternalInput").ap()
    w1_in = nc.dram_tensor("W1", [h_dim, IN], f32, kind="ExternalInput").ap()
    gamma_in = nc.dram_tensor("gamma", [h_dim], f32, kind="ExternalInput").ap()
    beta_in = nc.dram_tensor("beta", [h_dim], f32, kind="ExternalInput").ap()
    w2_in = nc.dram_tensor("W2", [OUT, h_dim], f32, kind="ExternalInput").ap()
    out_d = nc.dram_tensor("out", [b_sh, OUT], f32, kind="ExternalOutput").ap()

    with tile.TileContext(nc) as tc:
        for _rep in range(repeats):
            _emit(nc, tc, _rep, x_in, w1_in, gamma_in, beta_in, w2_in, out_d,
                  b_sh, h_dim, n_cores, nm, nbt, groups, group_size,
                  batch_total, use_collective)

    nc.compile()
    return nc


def _emit(nc, tc, rep, x_in, w1_in, gamma_in, beta_in, w2_in, out_d,
          b_sh, h_dim, n_cores, nm, nbt, groups, gs, batch_total,
          use_collective):
    with (
        tc.tile_pool(name=f"r{rep}const", bufs=1) as const,
        tc.tile_pool(name=f"r{rep}dram", bufs=1, space="DRAM") as dram,
    ):
        ident = const.tile([128, 128], f32)
        make_identity(nc, ident[:])
        ident16 = const.tile([128, 128], f16)
        nc.vector.tensor_copy(ident16[:], ident[:])
        identb = const.tile([128, 128], bf16)
        nc.vector.tensor_copy(identb[:], ident[:])
        sW2T = const.tile([128, nm, OUT], bf16)
        gamma_pm = const.tile([128, nm], f32)
        beta_pm = const.tile([128, nm], f32)
        scale_pm = const.tile([128, nm], f32)
        bias_pm = const.tile([128, nm], f32)
        stats = const.tile([128, nm, 4], f32)
        nc.vector.memset(stats[:], 0.0)

        w1bf_d = dram.tile([h_dim, KF + 128], bf16)

        with tc.tile_pool(name=f"r{rep}persist", bufs=1) as persist:
            xhiT = [persist.tile([128, b_sh], f16, name=f"xhiT{k}")
                    for k in range(KFULL)]
            xloT = [persist.tile([128, b_sh], bf16, name=f"xloT{k}")
                    for k in range(KFULL)]
            xmixT = persist.tile([128, b_sh], f16)
            sW1T = [persist.tile([128, h_dim], bf16, name=f"sW1T{k}")
                    for k in range(KFULL)]
            sW1mixT = persist.tile([128, h_dim], bf16)

            with (
                tc.tile_pool(name=f"r{rep}prolog", bufs=2) as prolog,
                tc.tile_pool(name=f"r{rep}prolog1", bufs=1) as prolog1,
                tc.tile_pool(name=f"r{rep}pps", bufs=7, space="PSUM") as pps,
            ):
                w2_sb = prolog1.tile([OUT, h_dim], f32, tag="w2sb")
                nc.scalar.dma_start(w2_sb[:], w2_in)
                for m in range(nm):
                    pt = pps.tile([128, OUT], f32, tag="pp")
                    nc.tensor.transpose(
                        pt[:], w2_sb[:OUT, m * 128:(m + 1) * 128],
                        ident[:OUT, :OUT])
                    nc.scalar.activation(sW2T[:, m, :], pt[:], AF.Sign)

                ga_sb = prolog1.tile([nm, 128], f32, tag="gasb")
                be_sb = prolog1.tile([nm, 128], f32, tag="besb")
                nc.scalar.dma_start(
                    ga_sb[:], gamma_in.rearrange("(m p) -> m p", p=128))
                nc.scalar.dma_start(
                    be_sb[:], beta_in.rearrange("(m p) -> m p", p=128))
                ga_ps = pps.tile([128, nm], f32, tag="pp")
                nc.tensor.transpose(ga_ps[:], ga_sb[:], ident[:nm, :nm])
                nc.scalar.copy(gamma_pm[:], ga_ps[:])
                be_ps = pps.tile([128, nm], f32, tag="pp")
                nc.tensor.transpose(be_ps[:], be_sb[:], ident[:nm, :nm])
                nc.scalar.copy(beta_pm[:], be_ps[:])

                NQ = 4
                xq = nbt // NQ
                wq = nm // NQ
                for q in range(NQ):
                    xt = prolog.tile([128, xq, IN], f32, tag="xt")
                    nc.sync.dma_start(
                        xt[:],
                        x_in[q * xq * 128:(q + 1) * xq * 128, :].rearrange(
                            "(t p) c -> p t c", p=128))
                    xhi = prolog.tile([128, xq, KF + 128], f16, tag="xhi")
                    xlo = prolog.tile([128, xq, KF], bf16, tag="xlo")
                    nc.vector.tensor_copy(xhi[:, :, :IN], xt[:])
                    nc.vector.tensor_tensor(
                        xlo[:], xt[:, :, :KF], xhi[:, :, :KF],
                        op=ALU.subtract)
                    nc.vector.tensor_tensor(
                        xhi[:, :, IN:IN + KTAIL], xt[:, :, KF:],
                        xhi[:, :, KF:IN], op=ALU.subtract)
                    nc.vector.memset(xhi[:, :, IN + KTAIL:], 0.0)
                    for ti in range(xq):
                        t = q * xq + ti
                        tcol = slice(t * 128, (t + 1) * 128)
                        for k in range(KFULL + 1):
                            pth = pps.tile([128, 128], f16, tag="pp")
                            nc.tensor.transpose(
                                pth[:], xhi[:, ti, k * 128:(k + 1) * 128],
                                ident16[:])
                            dst = xmixT if k == KFULL else xhiT[k]
                            nc.vector.tensor_copy(dst[:, tcol], pth[:])
                        for k in range(KFULL):
                            ptl = pps.tile([128, 128], bf16, tag="pp")
                            nc.tensor.transpose(
                                ptl[:], xlo[:, ti, k * 128:(k + 1) * 128],
                                identb[:])
                            nc.vector.tensor_copy(xloT[k][:, tcol], ptl[:])

                    wr = slice(q * wq * 128, (q + 1) * wq * 128)
                    nc.gpsimd.dma_start(w1bf_d[wr, :IN], w1_in[wr, :])
                    for k in range(KFULL):
                        nc.scalar.dma_start_transpose(
                            sW1T[k][:, wr], w1bf_d[wr, k * 128:(k + 1) * 128])
                    nc.scalar.dma_start_transpose(
                        sW1mixT[:, wr], w1bf_d[wr, KF:])

                nc.sync.dma_start(sW1mixT[16:32, :], sW1mixT[0:16, :])
                for wtile in sW1T:
                    nc.vector.tensor_scalar(
                        wtile[:], wtile[:], 0.0, None, op0=ALU.is_ge)
                    nc.vector.tensor_scalar(
                        wtile[:], wtile[:], 2.0, 1.0,
                        op0=ALU.mult, op1=ALU.subtract)
                nc.vector.tensor_scalar(
                    sW1mixT[0:32, :], sW1mixT[0:32, :], 0.0, None,
                    op0=ALU.is_ge)
                nc.vector.tensor_scalar(
                    sW1mixT[0:32, :], sW1mixT[0:32, :], 2.0, 1.0,
                    op0=ALU.mult, op1=ALU.subtract)
                nc.vector.memset(sW1mixT[32:64, :], 0.0)
                nc.vector.memset(sW1mixT[64:96, :], 0.0)
                nc.vector.memset(sW1mixT[96:128, :], 0.0)

            with (
                tc.tile_pool(name=f"r{rep}hwin", bufs=gs + 3) as hwin,
                tc.tile_pool(name=f"r{rep}sq", bufs=2) as sqp,
                tc.tile_pool(name=f"r{rep}sg", bufs=2) as sgp,
                tc.tile_pool(name=f"r{rep}gst", bufs=2) as gstp,
                tc.tile_pool(name=f"r{rep}ps1", bufs=2, space="PSUM") as ps1,
                tc.tile_pool(name=f"r{rep}ps2", bufs=1, space="PSUM") as ps2,
                tc.tile_pool(name=f"r{rep}ep", bufs=1) as ep,
            ):
                psL = ps2.tile([OUT, b_sh], f32, tag="psl")
                passes = (
                    [(sW1T[k], xhiT[k]) for k in range(KFULL)]
                    + [(sW1T[k], xloT[k]) for k in range(KFULL)]
                    + [(sW1mixT, xmixT)]
                )
                h_tiles = {}

                hsz = min(1024, b_sh)
                ncs = max(1, hsz // 512)
                csz = hsz // ncs
                for g, gms in enumerate(groups):
                    for m in gms:
                        h_sb = hwin.tile([128, b_sh], f32, tag="hsb")
                        h_tiles[m] = h_sb
                        for hf in range(b_sh // hsz):
                            ph = ps1.tile([128, hsz], f32, tag="ph")
                            for pi, (wt, xt_) in enumerate(passes):
                                lhsT = wt[:, m * 128:(m + 1) * 128]
                                for c in range(ncs):
                                    off = hf * hsz + c * csz
                                    nc.tensor.matmul(
                                        ph[:, c * csz:(c + 1) * csz],
                                        lhsT, xt_[:, off:off + csz],
                                        start=(pi == 0),
                                        stop=(pi == len(passes) - 1),
                                    )
                            nc.scalar.activation(
                                h_sb[:, hf * hsz:(hf + 1) * hsz], ph[:],
                                AF.Identity,
                                accum_out=stats[:, m, hf:hf + 1])
                            sq = sqp.tile([128, hsz], bf16, tag="sq")
                            nc.scalar.activation(
                                sq[:], ph[:], AF.Square,
                                accum_out=stats[:, m, 2 + hf:3 + hf])

                    g0, gn = gms[0], len(gms)
                    c_in = dram.tile([128, gn * 4], f32, name=f"cci{g}")
                    c_out = dram.tile([128, gn * 4], f32, name=f"cco{g}")
                    nc.sync.dma_start(
                        c_in[:], stats[:, g0:g0 + gn, :])
                    if use_collective:
                        nc.gpsimd.collective_compute(
                            "AllReduce", ALU.add,
                            replica_groups=[list(range(n_cores))],
                            ins=[c_in.opt()], outs=[c_out.opt()],
                        )
                    else:
                        nc.sync.dma_start(c_out[:], c_in[:])
                    gst = gstp.tile([128, gn, 4], f32, tag="gst")
                    nc.sync.dma_start(gst[:], c_out[:])

                    msl = slice(g0, g0 + gn)
                    mean_t = gstp.tile([128, gn], f32, tag="mean")
                    var_t = gstp.tile([128, gn], f32, tag="var")
                    tmp_t = gstp.tile([128, gn], f32, tag="tmp")
                    nc.vector.tensor_tensor(
                        mean_t[:], gst[:, :, 0], gst[:, :, 1], op=ALU.add)
                    nc.vector.tensor_scalar_mul(
                        mean_t[:], mean_t[:], 1.0 / batch_total)
                    nc.vector.tensor_tensor(
                        var_t[:], gst[:, :, 2], gst[:, :, 3], op=ALU.add)
                    nc.vector.tensor_scalar_mul(
                        var_t[:], var_t[:], 1.0 / batch_total)
                    nc.vector.tensor_tensor(
                        tmp_t[:], mean_t[:], mean_t[:], op=ALU.mult)
                    nc.vector.tensor_tensor(
                        var_t[:], var_t[:], tmp_t[:], op=ALU.subtract)
                    nc.vector.tensor_scalar_add(var_t[:], var_t[:], BN_EPS)
                    nc.vector.reciprocal(tmp_t[:], var_t[:])
                    nc.scalar.activation(tmp_t[:], tmp_t[:], AF.Sqrt)
                    nc.vector.tensor_tensor(
                        scale_pm[:, msl], tmp_t[:], gamma_pm[:, msl],
                        op=ALU.mult)
                    nc.vector.tensor_tensor(
                        tmp_t[:], mean_t[:], scale_pm[:, msl], op=ALU.mult)
                    nc.vector.tensor_tensor(
                        bias_pm[:, msl], beta_pm[:, msl], tmp_t[:],
                        op=ALU.subtract)

                    for m in gms:
                        s_t = sgp.tile([128, b_sh], bf16, tag="st")
                        nc.scalar.activation(
                            s_t[:], h_tiles.pop(m)[:], AF.Sign,
                            bias=bias_pm[:, m:m + 1],
                            scale=scale_pm[:, m:m + 1])
                        for c in range(b_sh // 512):
                            nc.tensor.matmul(
                                psL[:, c * 512:(c + 1) * 512],
                                sW2T[:, m:m + 1, :],
                                s_t[:, c * 512:(c + 1) * 512],
                                start=(m == 0), stop=(m == nm - 1),
                            )

                LT = ep.tile([OUT, b_sh], f32)
                nc.scalar.copy(LT[:], psL[:])
                psT = ps2.tile([128, nbt * OUT], f32, tag="psl")
                for t in range(nbt):
                    nc.tensor.transpose(
                        psT[:, t * OUT:(t + 1) * OUT],
                        LT[:OUT, t * 128:(t + 1) * 128],
                        ident[:OUT, :OUT])
                Lb = ep.tile([128, nbt, OUT], f32)
                nc.scalar.copy(Lb[:], psT[:])

                negmax = ep.tile([128, nbt], f32)
                nc.vector.tensor_reduce(
                    negmax[:], Lb[:], axis=mybir.AxisListType.X,
                    op=ALU.max, negate=True)
                shifted = ep.tile([128, nbt, OUT], f32)
                nc.vector.tensor_tensor(
                    shifted[:], Lb[:],
                    negmax[:][:, :, None].broadcast_to([128, nbt, OUT]),
                    op=ALU.add)
                expv = ep.tile([128, nbt, OUT], f32)
                nc.scalar.activation(expv[:], shifted[:], AF.Exp)
                sumexp = ep.tile([128, nbt], f32)
                nc.vector.tensor_reduce(
                    sumexp[:], expv[:], axis=mybir.AxisListType.X, op=ALU.add)
                lse = ep.tile([128, nbt], f32)
                nc.scalar.activation(lse[:], sumexp[:], AF.Ln)
                lsm = ep.tile([128, nbt, OUT], f32)
                nc.vector.tensor_tensor(
                    lsm[:], shifted[:],
                    lse[:][:, :, None].broadcast_to([128, nbt, OUT]),
                    op=ALU.subtract)
                nc.sync.dma_start(
                    out_d.rearrange("(t p) o -> p t o", p=128), lsm[:])


_NC_CACHE = {}


def _get_nc():
    if "nc" not in _NC_CACHE:
        _NC_CACHE["nc"] = build_nc()
    return _NC_CACHE["nc"]


def kernel(x, W1, gamma, beta, W2):
    x = np.ascontiguousarray(np.asarray(x), dtype=np.float32)
    W1 = np.ascontiguousarray(np.asarray(W1), dtype=np.float32)
    gamma = np.ascontiguousarray(np.asarray(gamma), dtype=np.float32)
    beta = np.ascontiguousarray(np.asarray(beta), dtype=np.float32)
    W2 = np.ascontiguousarray(np.asarray(W2), dtype=np.float32)

    nc = _get_nc()
    b_sh = B // N_CORES
    in_maps = [
        {
            "x": x[c * b_sh:(c + 1) * b_sh],
            "W1": W1,
            "gamma": gamma,
            "beta": beta,
            "W2": W2,
        }
        for c in range(N_CORES)
    ]
    res = bass_utils.run_bass_kernel_spmd(
        nc, in_maps, core_ids=list(range(N_CORES)))
    return np.concatenate(
        [res.results[c]["out"] for c in range(N_CORES)], axis=0)


# revision 28
# speedup vs baseline: 1.1004x; 1.0141x over previous
import sys

if "/opt/trn_rl_repo" not in sys.path:
    sys.path.insert(0, "/opt/trn_rl_repo")

import numpy as np

import concourse.mybir as mybir
import concourse.tile as tile
from concourse import bacc, bass_utils
from concourse.masks import make_identity

N_CORES = 8
B, IN, H, OUT = 16384, 784, 4096, 10
BN_EPS = 1e-5
KFULL = 6
KF = KFULL * 128
KTAIL = IN - KF

f32 = mybir.dt.float32
bf16 = mybir.dt.bfloat16
f16 = mybir.dt.float16
AF = mybir.ActivationFunctionType
ALU = mybir.AluOpType


def build_nc(b_sh=B // N_CORES, h_dim=H, n_cores=N_CORES, use_collective=True,
             group_size=4, repeats=1):
    nm = h_dim // 128
    nbt = b_sh // 128
    groups = []
    mstart = 0
    while mstart < nm:
        g_sz = min(group_size, nm - mstart)
        if nm - mstart == group_size and group_size >= 4:
            groups.append(list(range(mstart, mstart + g_sz // 2)))
            groups.append(list(range(mstart + g_sz // 2, mstart + g_sz)))
        else:
            groups.append(list(range(mstart, mstart + g_sz)))
        mstart += g_sz
    batch_total = b_sh * n_cores if use_collective else b_sh

    nc = bacc.Bacc("TRN2", target_bir_lowering=False, debug=False,
                   num_devices=n_cores)

    x_in = nc.dram_tensor("x", [b_sh, IN], f32, kind="ExternalInput").ap()
    w1_in = nc.dram_tensor("W1", [h_dim, IN], f32, kind="ExternalInput").ap()
    gamma_in = nc.dram_tensor("gamma", [h_dim], f32, kind="ExternalInput").ap()
    beta_in = nc.dram_tensor("beta", [h_dim], f32, kind="ExternalInput").ap()
    w2_in = nc.dram_tensor("W2", [OUT, h_dim], f32, kind="ExternalInput").ap()
    out_d = nc.dram_tensor("out", [b_sh, OUT], f32, kind="ExternalOutput").ap()

    with tile.TileContext(nc) as tc:
        for _rep in range(repeats):
            _emit(nc, tc, _rep, x_in, w1_in, gamma_in, beta_in, w2_in, out_d,
                  b_sh, h_dim, n_cores, nm, nbt, groups, group_size,
                  batch_total, use_collective)

    nc.compile()
    return nc


def _emit(nc, tc, rep, x_in, w1_in, gamma_in, beta_in, w2_in, out_d,
          b_sh, h_dim, n_cores, nm, nbt, groups, gs, batch_total,
          use_collective):
    with (
        tc.tile_pool(name=f"r{rep}const", bufs=1) as const,
        tc.tile_pool(name=f"r{rep}dram", bufs=1, space="DRAM") as dram,
    ):
        ident = const.tile([128, 128], f32)
        make_identity(nc, ident[:])
        ident16 = const.tile([128, 128], f16)
        nc.vector.tensor_copy(ident16[:], ident[:])
        identb = const.tile([128, 128], bf16)
        nc.vector.tensor_copy(identb[:], ident[:])
        sW2T = const.tile([128, nm, OUT], bf16)
        gamma_pm = const.tile([128, nm], f32)
        beta_pm = const.tile([128, nm], f32)
        scale_pm = const.tile([128, nm], f32)
        bias_pm = const.tile([128, nm], f32)
        stats = const.tile([128, nm, 4], f32)
        nc.vector.memset(stats[:], 0.0)

        w1bf_d = dram.tile([h_dim, KF + 128], bf16)

        with tc.tile_pool(name=f"r{rep}persist", bufs=1) as persist:
            xhiT = [persist.tile([128, b_sh], f16, name=f"xhiT{k}")
                    for k in range(KFULL)]
            xloT = [persist.tile([128, b_sh], bf16, name=f"xloT{k}")
                    for k in range(KFULL)]
            xmixT = persist.tile([128, b_sh], f16)
            sW1T = [persist.tile([128, h_dim], bf16, name=f"sW1T{k}")
                    for k in range(KFULL)]
            sW1mixT = persist.tile([128, h_dim], bf16)

            with (
                tc.tile_pool(name=f"r{rep}prolog", bufs=2) as prolog,
                tc.tile_pool(name=f"r{rep}prolog1", bufs=1) as prolog1,
                tc.tile_pool(name=f"r{rep}pps", bufs=7, space="PSUM") as pps,
            ):
                w2_sb = prolog1.tile([OUT, h_dim], f32, tag="w2sb")
                nc.scalar.dma_start(w2_sb[:], w2_in)
                for m in range(nm):
                    pt = pps.tile([128, OUT], f32, tag="pp")
                    nc.tensor.transpose(
                        pt[:], w2_sb[:OUT, m * 128:(m + 1) * 128],
                        ident[:OUT, :OUT])
                    nc.scalar.activation(sW2T[:, m, :], pt[:], AF.Sign)

                ga_sb = prolog1.tile([nm, 128], f32, tag="gasb")
                be_sb = prolog1.tile([nm, 128], f32, tag="besb")
                nc.scalar.dma_start(
                    ga_sb[:], gamma_in.rearrange("(m p) -> m p", p=128))
                nc.scalar.dma_start(
                    be_sb[:], beta_in.rearrange("(m p) -> m p", p=128))
                ga_ps = pps.tile([128, nm], f32, tag="pp")
                nc.tensor.transpose(ga_ps[:], ga_sb[:], ident[:nm, :nm])
                nc.scalar.copy(gamma_pm[:], ga_ps[:])
                be_ps = pps.tile([128, nm], f32, tag="pp")
                nc.tensor.transpose(be_ps[:], be_sb[:], ident[:nm, :nm])
                nc.scalar.copy(beta_pm[:], be_ps[:])

                NQ = 4
                xq = nbt // NQ
                wq = nm // NQ
                for q in range(NQ):
                    xt = prolog.tile([128, xq, IN], f32, tag="xt")
                    nc.sync.dma_start(
                        xt[:],
                        x_in[q * xq * 128:(q + 1) * xq * 128, :].rearrange(
                            "(t p) c -> p t c", p=128))
                    xhi = prolog.tile([128, xq, KF + 128], f16, tag="xhi")
                    xlo = prolog.tile([128, xq, KF], bf16, tag="xlo")
                    nc.vector.tensor_copy(xhi[:, :, :IN], xt[:])
                    nc.vector.tensor_tensor(
                        xlo[:], xt[:, :, :KF], xhi[:, :, :KF],
                        op=ALU.subtract)
                    nc.vector.tensor_tensor(
                        xhi[:, :, IN:IN + KTAIL], xt[:, :, KF:],
                        xhi[:, :, KF:IN], op=ALU.subtract)
                    nc.vector.memset(xhi[:, :, IN + KTAIL:], 0.0)
                    for ti in range(xq):
                        t = q * xq + ti
                        tcol = slice(t * 128, (t + 1) * 128)
                        for k in range(KFULL + 1):
                            pth = pps.tile([128, 128], f16, tag="pp")
                            nc.tensor.transpose(
                                pth[:], xhi[:, ti, k * 128:(k + 1) * 128],
                                ident16[:])
                            dst = xmixT if k == KFULL else xhiT[k]
                            nc.vector.tensor_copy(dst[:, tcol], pth[:])
                        for k in range(KFULL):
                            ptl = pps.tile([128, 128], bf16, tag="pp")
                            nc.tensor.transpose(
                                ptl[:], xlo[:, ti, k * 128:(k + 1) * 128],
                                identb[:])
                            nc.vector.tensor_copy(xloT[k][:, tcol], ptl[:])

                    wr = slice(q * wq * 128, (q + 1) * wq * 128)
                    nc.gpsimd.dma_start(w1bf_d[wr, :IN], w1_in[wr, :])
                    for k in range(KFULL):
                        nc.scalar.dma_start_transpose(
                            sW1T[k][:, wr], w1bf_d[wr, k * 128:(k + 1) * 128])
                    nc.scalar.dma_start_transpose(
                        sW1mixT[:, wr], w1bf_d[wr, KF:])

                nc.sync.dma_start(sW1mixT[16:32, :], sW1mixT[0:16, :])
                for wtile in sW1T:
                    nc.vector.tensor_scalar(
                        wtile[:], wtile[:], 0.0, None, op0=ALU.is_ge)
                    nc.vector.tensor_scalar(
                        wtile[:], wtile[:], 2.0, 1.0,
                        op0=ALU.mult, op1=ALU.subtract)
                nc.vector.tensor_scalar(
                    sW1mixT[0:32, :], sW1mixT[0:32, :], 0.0, None,
                    op0=ALU.is_ge)
                nc.vector.tensor_scalar(
                    sW1mixT[0:32, :], sW1mixT[0:32, :], 2.0, 1.0,
                    op0=ALU.mult, op1=ALU.subtract)
                nc.vector.memset(sW1mixT[32:64, :], 0.0)
                nc.vector.memset(sW1mixT[64:96, :], 0.0)
                nc.vector.memset(sW1mixT[96:128, :], 0.0)

            with (
                tc.tile_pool(name=f"r{rep}hwin", bufs=gs + 4) as hwin,
                tc.tile_pool(name=f"r{rep}sq", bufs=2) as sqp,
                tc.tile_pool(name=f"r{rep}sg", bufs=3) as sgp,
                tc.tile_pool(name=f"r{rep}gst", bufs=2) as gstp,
                tc.tile_pool(name=f"r{rep}ps1", bufs=2, space="PSUM") as ps1,
                tc.tile_pool(name=f"r{rep}ps2", bufs=1, space="PSUM") as ps2,
                tc.tile_pool(name=f"r{rep}ep", bufs=1) as ep,
            ):
                psL = ps2.tile([OUT, b_sh], f32, tag="psl")
                passes = (
                    [(sW1T[k], xhiT[k]) for k in range(KFULL)]
                    + [(sW1T[k], xloT[k]) for k in range(KFULL)]
                    + [(sW1mixT, xmixT)]
                )
                h_tiles = {}

                hsz = min(1024, b_sh)
                ncs = max(1, hsz // 512)
                csz = hsz // ncs
                for g, gms in enumerate(groups):
                    for m in gms:
                        h_sb = hwin.tile([128, b_sh], f32, tag="hsb")
                        h_tiles[m] = h_sb
                        for hf in range(b_sh // hsz):
                            ph = ps1.tile([128, hsz], f32, tag="ph")
                            for pi, (wt, xt_) in enumerate(passes):
                                lhsT = wt[:, m * 128:(m + 1) * 128]
                                for c in range(ncs):
                                    off = hf * hsz + c * csz
                                    nc.tensor.matmul(
                                        ph[:, c * csz:(c + 1) * csz],
                                        lhsT, xt_[:, off:off + csz],
                                        start=(pi == 0),
                                        stop=(pi == len(passes) - 1),
                                    )
                            nc.scalar.activation(
                                h_sb[:, hf * hsz:(hf + 1) * hsz], ph[:],
                                AF.Identity,
                                accum_out=stats[:, m, hf:hf + 1])
                            sq = sqp.tile([128, hsz], bf16, tag="sq")
                            nc.scalar.activation(
                                sq[:], ph[:], AF.Square,
                                accum_out=stats[:, m, 2 + hf:3 + hf])

                    g0, gn = gms[0], len(gms)
                    c_in = dram.tile([128, gn * 4], f32, name=f"cci{g}")
                    c_out = dram.tile([128, gn * 4], f32, name=f"cco{g}")
                    nc.sync.dma_start(
                        c_in[:], stats[:, g0:g0 + gn, :])
                    if use_collective:
                        nc.gpsimd.collective_compute(
                            "AllReduce", ALU.add,
                            replica_groups=[list(range(n_cores))],
                            ins=[c_in.opt()], outs=[c_out.opt()],
                        )
                    else:
                        nc.sync.dma_start(c_out[:], c_in[:])
                    gst = gstp.tile([128, gn, 4], f32, tag="gst")
                    nc.sync.dma_start(gst[:], c_out[:])

                    msl = slice(g0, g0 + gn)
                    mean_t = gstp.tile([128, gn], f32, tag="mean")
                    var_t = gstp.tile([128, gn], f32, tag="var")
                    tmp_t = gstp.tile([128, gn], f32, tag="tmp")
                    nc.vector.tensor_tensor(
                        mean_t[:], gst[:, :, 0], gst[:, :, 1], op=ALU.add)
                    nc.vector.tensor_scalar_mul(
                        mean_t[:], mean_t[:], 1.0 / batch_total)
                    nc.vector.tensor_tensor(
                        var_t[:], gst[:, :, 2], gst[:, :, 3], op=ALU.add)
                    nc.vector.tensor_scalar_mul(
                        var_t[:], var_t[:], 1.0 / batch_total)
                    nc.vector.tensor_tensor(
                        tmp_t[:], mean_t[:], mean_t[:], op=ALU.mult)
                    nc.vector.tensor_tensor(
                        var_t[:], var_t[:], tmp_t[:], op=ALU.subtract)
                    nc.vector.tensor_scalar_add(var_t[:], var_t[:], BN_EPS)
                    nc.vector.reciprocal(tmp_t[:], var_t[:])
                    nc.scalar.activation(tmp_t[:], tmp_t[:], AF.Sqrt)
                    nc.vector.tensor_tensor(
                        scale_pm[:, msl], tmp_t[:], gamma_pm[:, msl],
                        op=ALU.mult)
                    nc.vector.tensor_tensor(
                        tmp_t[:], mean_t[:], scale_pm[:, msl], op=ALU.mult)
                    nc.vector.tensor_tensor(
                        bias_pm[:, msl], beta_pm[:, msl], tmp_t[:],
                        op=ALU.subtract)

                    for m in gms:
                        s_t = sgp.tile([128, b_sh], bf16, tag="st")
                        nc.scalar.activation(
                            s_t[:], h_tiles.pop(m)[:], AF.Sign,
                            bias=bias_pm[:, m:m + 1],
                            scale=scale_pm[:, m:m + 1])
                        for c in range(b_sh // 512):
                            nc.tensor.matmul(
                                psL[:, c * 512:(c + 1) * 512],
                                sW2T[:, m:m + 1, :],
                                s_t[:, c * 512:(c + 1) * 512],
                                start=(m == 0), stop=(m == nm - 1),
                            )

                LT = ep.tile([OUT, b_sh], f32)
                nc.scalar.copy(LT[:], psL[:])
                psT = ps2.tile([128, nbt * OUT], f32, tag="psl")
                for t in range(nbt):
                    nc.tensor.transpose(
                        psT[:, t * OUT:(t + 1) * OUT],
                        LT[:OUT, t * 128:(t + 1) * 128],
                        ident[:OUT, :OUT])
                Lb = ep.tile([128, nbt, OUT], f32)
                nc.scalar.copy(Lb[:], psT[:])

                negmax = ep.tile([128, nbt], f32)
                nc.vector.tensor_reduce(
                    negmax[:], Lb[:], axis=mybir.AxisListType.X,
                    op=ALU.max, negate=True)
                shifted = ep.tile([128, nbt, OUT], f32)
                nc.vector.tensor_tensor(
                    shifted[:], Lb[:],
                    negmax[:][:, :, None].broadcast_to([128, nbt, OUT]),
                    op=ALU.add)
                expv = ep.tile([128, nbt, OUT], f32)
                nc.scalar.activation(expv[:], shifted[:], AF.Exp)
                sumexp = ep.tile([128, nbt], f32)
                nc.vector.tensor_reduce(
                    sumexp[:], expv[:], axis=mybir.AxisListType.X, op=ALU.add)
                lse = ep.tile([128, nbt], f32)
                nc.scalar.activation(lse[:], sumexp[:], AF.Ln)
                lsm = ep.tile([128, nbt, OUT], f32)
                nc.vector.tensor_tensor(
                    lsm[:], shifted[:],
                    lse[:][:, :, None].broadcast_to([128, nbt, OUT]),
                    op=ALU.subtract)
                nc.sync.dma_start(
                    out_d.rearrange("(t p) o -> p t o", p=128), lsm[:])


_NC_CACHE = {}


def _get_nc():
    if "nc" not in _NC_CACHE:
        _NC_CACHE["nc"] = build_nc()
    return _NC_CACHE["nc"]


def kernel(x, W1, gamma, beta, W2):
    x = np.ascontiguousarray(np.asarray(x), dtype=np.float32)
    W1 = np.ascontiguousarray(np.asarray(W1), dtype=np.float32)
    gamma = np.ascontiguousarray(np.asarray(gamma), dtype=np.float32)
    beta = np.ascontiguousarray(np.asarray(beta), dtype=np.float32)
    W2 = np.ascontiguousarray(np.asarray(W2), dtype=np.float32)

    nc = _get_nc()
    b_sh = B // N_CORES
    in_maps = [
        {
            "x": x[c * b_sh:(c + 1) * b_sh],
            "W1": W1,
            "gamma": gamma,
            "beta": beta,
            "W2": W2,
        }
        for c in range(N_CORES)
    ]
    res = bass_utils.run_bass_kernel_spmd(
        nc, in_maps, core_ids=list(range(N_CORES)))
    return np.concatenate(
        [res.results[c]["out"] for c in range(N_CORES)], axis=0)
